# revision 1
# baseline (speedup 1.0000x reference)
"""Trainium2 Bass kernel for nn_ANN_Comp_29240137351521 (dense_cnn).

Reference computes, per batch row b of x [16384, 512] (complex, given as
real/imag f32 pairs):
    h = x @ w0                      # [B, 512] complex
    a = ifft(fft(h, n=1023)^2)      # full self-convolution, [B, 1023]
    out = |a @ wlast|               # [B, 10] f32

Algebraic collapse used here: the self-convolution + final contraction is a
polynomial-evaluation identity. With L = 1024 >= 2*512-1 evaluation points at
the L-th roots of unity:
    e   = x @ F        where F  = fft(w0, n=L, axis=1)        [512, L]
    z   = (e*e) @ Wt   where Wt = ifft(pad(wlast, L), axis=0) [L, 10]
    out = |z|
so the whole network is two dense matmuls + an elementwise complex square --
no FFT on device. F and Wt are tiny weight transforms folded on the host.

Real-expanded form on device (per core, data-parallel over batch), Gauss
3-multiplication split of the complex matmul; everything transposed (l on
partitions, batch free):
    P1 = xr@Fr ; P2 = xi@Fi ; P3 = (xr+xi)@(Fr+Fi)    (PSUM accumulation)
    m = 2*P1-P3 = er-ei ;  p = P3-2*P2 = er+ei        (DVE fused ops)
    s = p*m  = Re e^2                                 (DVE mult, bf16)
    a = p^2 ; b = m^2  (ACT squares, bf16)
    t = a-b  = 4*er*ei = 2*Im e^2                     (DVE 16-bit subtract)
    z += s@[Wtr|Wti] + t@[-Wti/2|Wtr/2]               (second matmul, one
                                                       accumulation chain
                                                       per batch chunk)
    host: out = sqrt(zr^2 + zi^2)

vs the previous revision: the z-stage uses 2 streams instead of 3 (-32 PE
matmuls, the a-b subtract moved to the DVE), z-weights are zero-padded to
128 columns so Fast Weight Load stays on (20-col LDWEIGHTS stalls the PE
~85ns each), x is packed contiguous per batch chunk for fat DMA
descriptors, the critical first chunks race in first on both HWDGE queues
while bulk x for later batch chunks goes through the software DGE ring
gated behind mid-stream compute tiles (a bare dma_start is scheduled
immediately and steals HBM bandwidth from the critical loads -- measured
as the f1 head chunk taking 5.7us instead of ~1us).

Sharding: pure data parallel -- batch split 8 ways, weights replicated.
"""

import numpy as np
import ml_dtypes

import concourse.bass as bass
import concourse.mybir as mybir
from concourse import bacc, tile
from concourse.bass_utils import run_bass_kernel_spmd

NCORES = 8
B, D, L, C = 16384, 512, 1024, 10
BC = B // NCORES
P = 128
BN = 512
ND = D // P
NL = L // P
NB = BC // BN

F32 = mybir.dt.float32
BF16 = mybir.dt.bfloat16
ALU = mybir.AluOpType

_NC_CACHE = None


def build_nc():
    global _NC_CACHE
    if _NC_CACHE is not None:
        return _NC_CACHE

    nc = bacc.Bacc(None, target_bir_lowering=False)

    xtr_d = nc.declare_dram_parameter("xT_r", [P, NB, ND * BN], BF16,
                                      isOutput=False)
    xti_d = nc.declare_dram_parameter("xT_i", [P, NB, ND * BN], BF16,
                                      isOutput=False)
    xts_d = nc.declare_dram_parameter("xT_s", [P, NB, ND * BN], BF16,
                                      isOutput=False)
    f1_d = nc.declare_dram_parameter("F_1", [P, ND * L], BF16, isOutput=False)
    f2_d = nc.declare_dram_parameter("F_2", [P, ND * L], BF16, isOutput=False)
    f3_d = nc.declare_dram_parameter("F_3", [P, ND * L], BF16, isOutput=False)
    # z-weights padded to 128 cols per l-chunk: NumWeights==128 keeps Fast
    # Weight Load enabled so the LDWEIGHTS overlaps the previous matmul's
    # stream (20-col loads stall the PE ~85ns each); the zero columns land
    # in PSUM rows 20..127 and are simply never read.
    w1_d = nc.declare_dram_parameter("W_1", [P, NL * P], BF16, isOutput=False)
    w2_d = nc.declare_dram_parameter("W_2", [P, NL * P], BF16, isOutput=False)
    out_d = nc.declare_dram_parameter("out", [2 * C, BC], F32, isOutput=True)

    with tile.TileContext(nc) as tc:
        with (
            tc.tile_pool(name="wts", bufs=1) as wts,
            tc.tile_pool(name="xs", bufs=1) as xs,
            tc.tile_pool(name="tmp", bufs=3) as tmp,
            tc.tile_pool(name="sqf", bufs=3) as sqf,
            tc.tile_pool(name="sq", bufs=5) as sq,
            tc.tile_pool(name="zo", bufs=2) as zo,
            tc.tile_pool(name="pse", bufs=2, space="PSUM") as pse,
            tc.tile_pool(name="psz", bufs=2, space="PSUM") as psz,
        ):
            # PE warm-up during the load phase (releases the HAM clock gate)
            dummy = wts.tile([P, 64], BF16, tag="dummy")
            nc.gpsimd.memset(dummy[:], 0.0)
            wacc = pse.tile([64, 64], F32, tag="p1")
            for i in range(40):
                nc.tensor.matmul(wacc[:], dummy[:, 0:64], dummy[:],
                                 start=(i == 0), stop=False,
                                 skip_group_check=True)

            def warm_fill(n):
                for _ in range(n):
                    nc.tensor.matmul(wacc[:], dummy[:, 0:64], dummy[:],
                                     start=False, stop=False,
                                     skip_group_check=True)

            f1 = wts.tile([P, ND * L], BF16, tag="f1")
            f2 = wts.tile([P, ND * L], BF16, tag="f2")
            f3 = wts.tile([P, ND * L], BF16, tag="f3")
            xtr = xs.tile([P, NB, ND * BN], BF16, tag="xtr")
            xti = xs.tile([P, NB, ND * BN], BF16, tag="xti")
            xts = xs.tile([P, NB, ND * BN], BF16, tag="xts")
            w1 = wts.tile([P, NL * P], BF16, tag="w1")
            w2 = wts.tile([P, NL * P], BF16, tag="w2")

            def flc(l):         # one l-chunk of F (l-major): 128KB
                return slice(l * D, (l + 1) * D)

            def fpair(k):       # two l-chunks of F: 256KB per DMA
                return slice(k * 2 * D, (k + 1) * 2 * D)

            # DMA order: tiny z-weights + first l-chunk of F + first batch of
            # x (split in d-halves) race in first on both hardware queues so
            # the PE can start ~3us in; everything else streams behind in fat
            # contiguous chunks (x is packed so [:, b, :] is 4KB/partition).
            # ~6.1MB (weights + F + x b0,b1) is needed within the first ~26us;
            # the two HWDGE queues sustain ~150GB/s each, so bulk x (b2/b3 +
            # one b1 stream) goes through the software DGE ring instead
            # (dispatch on the otherwise-idle Pool engine, same SDMA pool).
            def ftail(a, b):    # l-chunks [a, b) as one fat DMA
                return slice(a * D, b * D)
            nc.sync.dma_start(f1[:, flc(0)], f1_d[:, flc(0)])
            nc.scalar.dma_start(f2[:, flc(0)], f2_d[:, flc(0)])
            nc.sync.dma_start(xtr[:, 0, 0:2 * BN], xtr_d[:, 0, 0:2 * BN])
            nc.scalar.dma_start(xti[:, 0, 0:2 * BN], xti_d[:, 0, 0:2 * BN])
            nc.sync.dma_start(f3[:, flc(0)], f3_d[:, flc(0)])
            nc.scalar.dma_start(xts[:, 0, 0:2 * BN], xts_d[:, 0, 0:2 * BN])
            nc.sync.dma_start(xtr[:, 0, 2 * BN:], xtr_d[:, 0, 2 * BN:])
            nc.scalar.dma_start(xti[:, 0, 2 * BN:], xti_d[:, 0, 2 * BN:])
            nc.sync.dma_start(xts[:, 0, 2 * BN:], xts_d[:, 0, 2 * BN:])
            nc.gpsimd.dma_start(xts[:, 1, :], xts_d[:, 1, :])
            nc.sync.dma_start(f1[:, flc(1)], f1_d[:, flc(1)])
            nc.scalar.dma_start(f2[:, flc(1)], f2_d[:, flc(1)])
            nc.scalar.dma_start(f3[:, flc(1)], f3_d[:, flc(1)])
            nc.sync.dma_start(w1[:], w1_d[:])
            nc.scalar.dma_start(w2[:], w2_d[:])
            nc.sync.dma_start(f1[:, ftail(2, 8)], f1_d[:, ftail(2, 8)])
            nc.scalar.dma_start(f2[:, ftail(2, 8)], f2_d[:, ftail(2, 8)])
            nc.sync.dma_start(f3[:, ftail(2, 5)], f3_d[:, ftail(2, 5)])
            nc.scalar.dma_start(f3[:, ftail(5, 8)], f3_d[:, ftail(5, 8)])

            def late_x(bstreams, dep):
                # Delay the software-DGE dispatch of bulk x until `dep` (a
                # mid-stream compute tile) exists. A bare dma_start has no
                # dependencies and gets scheduled immediately, stealing HBM
                # bandwidth from the critical F/b0 loads in the first ~20us
                # -- so write a dep-gated byte into each destination slice
                # first; the WAW ordering paces the DMA.
                for xt, xd, b in bstreams:
                    nc.gpsimd.tensor_copy(xt[:, b, 0:1], dep)
                    nc.gpsimd.dma_start(xt[:, b, :], xd[:, b, :])

            def fsl(d, l):      # F weight chunk (d, l) in l-major packing
                return slice(l * D + d * P, l * D + (d + 1) * P)

            def wsl(l):
                return slice(l * P, (l + 1) * P)

            def dsl(d):
                return slice(d * BN, (d + 1) * BN)

            # z-matmuls run late (pending) so the PE never waits on DVE;
            # one 16-matmul accumulation chain per batch into rows 0..19 of
            # one PSUM bank (no column-tiling mode switches), copied out and
            # DMA'd at batch end.
            pending = []   # (zz, b, wt, wslice, rhs, bs)
            zcnt = {}      # b -> completed matmul count

            def zflush(batch):
                for (pzz, bkey, wt, ws, rhs, bs) in batch:
                    n = zcnt.get(bkey, 0)
                    zcnt[bkey] = n + 1
                    stop = (n == NL * 2 - 1)
                    nc.tensor.matmul(
                        pzz[:, :], wt[:, ws], rhs[:],
                        start=(n == 0), stop=stop,
                        skip_group_check=True)
                    if stop:
                        zt = zo.tile([2 * C, BN], F32, tag="zt")
                        nc.scalar.copy(zt[:], pzz[0:2 * C, :])
                        nc.sync.dma_start(out_d[:, bs], zt[:])

            warm_fill(50)
            for b in range(NB):
                bs = slice(b * BN, (b + 1) * BN)
                zz = psz.tile([P, BN], F32, tag="zz")
                for l in range(NL):
                    if b == 0 and l < 3:
                        warm_fill(8)
                    p1 = pse.tile([P, BN], F32, tag="p1")
                    p2 = pse.tile([P, BN], F32, tag="p2")
                    p3 = pse.tile([P, BN], F32, tag="p3")
                    for d in range(ND):
                        nc.tensor.matmul(
                            p1[:], f1[:, fsl(d, l)], xtr[:, b, dsl(d)],
                            start=(d == 0), stop=(d == ND - 1),
                            skip_group_check=True)
                    for d in range(ND):
                        nc.tensor.matmul(
                            p2[:], f2[:, fsl(d, l)], xti[:, b, dsl(d)],
                            start=(d == 0), stop=(d == ND - 1),
                            skip_group_check=True)
                    for d in range(ND):
                        nc.tensor.matmul(
                            p3[:], f3[:, fsl(d, l)], xts[:, b, dsl(d)],
                            start=(d == 0), stop=(d == ND - 1),
                            skip_group_check=True)

                    if len(pending) >= (2 if b == NB - 1 else 6):
                        zflush(pending[:2])
                        pending = pending[2:]

                    # c3 = P3 (ACT copy to SBUF -- DVE stt can't take two
                    # PSUM operands) ; m = 2*P1 - c3 ; p = c3 - 2*P2  (DVE)
                    c3 = tmp.tile([P, BN], F32, tag="c3")
                    nc.scalar.copy(c3[:], p3[:])
                    m = tmp.tile([P, BN], F32, tag="m")
                    nc.vector.scalar_tensor_tensor(
                        m[:], p1[:], 2.0, c3[:], ALU.mult, ALU.subtract)
                    p = tmp.tile([P, BN], F32, tag="p")
                    nc.vector.scalar_tensor_tensor(
                        p[:], p2[:], -2.0, c3[:], ALU.mult, ALU.add)
                    # s = p*m = Re e^2 (DVE); a = p^2, bq = m^2 (ACT, bf16);
                    # t = a - bq = 2*Im e^2 (DVE, 16-bit 2x mode)
                    s = sq.tile([P, BN], BF16, tag="s")
                    nc.vector.tensor_mul(s[:], p[:], m[:])
                    a = sqf.tile([P, BN], BF16, tag="a")
                    nc.scalar.square(a[:], p[:])
                    bq = sqf.tile([P, BN], BF16, tag="bq")
                    nc.scalar.square(bq[:], m[:])
                    t = sq.tile([P, BN], BF16, tag="t")
                    nc.vector.tensor_sub(t[:], a[:], bq[:])

                    if b == 0 and l == 1:
                        late_x([(xtr, xtr_d, 1), (xti, xti_d, 1)], s[:, 0:1])
                    elif b == 0 and l == 4:
                        late_x([(xtr, xtr_d, 2), (xti, xti_d, 2),
                                (xts, xts_d, 2)], s[:, 0:1])
                    elif b == 1 and l == 4:
                        late_x([(xtr, xtr_d, 3), (xti, xti_d, 3),
                                (xts, xts_d, 3)], s[:, 0:1])

                    for wt, rhs in ((w1, s), (w2, t)):
                        pending.append((zz, b, wt, wsl(l), rhs, bs))

            while pending:
                zflush(pending[:2])
                pending = pending[2:]

    nc.compile()
    _NC_CACHE = nc
    return nc


def _pack128(arr):
    R = arr.shape[0] // P
    return np.ascontiguousarray(
        arr.reshape(R, P, arr.shape[1]).transpose(1, 0, 2).reshape(P, -1))


def _packF(a):
    """[512, 1024] -> [128, 4096] l-major: col l*512 + d*128 + c holds
    F[d*128+p, l*128+c], so one l-chunk's 4 contraction slices are
    contiguous and can be DMA'd just ahead of their first use."""
    return np.ascontiguousarray(
        a.reshape(ND, P, NL, P).transpose(1, 2, 0, 3).reshape(P, -1))


def _host_weights(w0_real, w0_imag, wlast_real, wlast_imag):
    w0 = w0_real.astype(np.float64) + 1j * w0_imag.astype(np.float64)
    wl = wlast_real.astype(np.float64) + 1j * wlast_imag.astype(np.float64)
    F = np.fft.fft(w0, n=L, axis=1)
    Wt = np.fft.ifft(
        np.concatenate([wl, np.zeros((1, C))], axis=0), axis=0)
    bf = ml_dtypes.bfloat16
    F1 = _packF(F.real.astype(bf))
    F2 = _packF(F.imag.astype(bf))
    F3 = _packF((F.real + F.imag).astype(bf))
    Wtr, Wti = Wt.real, Wt.imag

    def padw(a):        # [1024, 20] -> [1024, 128] zero-padded
        return np.concatenate(
            [a, np.zeros((a.shape[0], P - a.shape[1]))], axis=1)

    W1 = _pack128(padw(np.hstack([Wtr, Wti])).astype(bf))
    W2 = _pack128(padw(np.hstack([-Wti, Wtr]) / 2.0).astype(bf))
    return F1, F2, F3, W1, W2


def make_in_maps(x_real, x_imag, w0_real, w0_imag, wlast_real, wlast_imag):
    F1, F2, F3, W1, W2 = _host_weights(
        w0_real, w0_imag, wlast_real, wlast_imag)
    bf = ml_dtypes.bfloat16
    xr = np.ascontiguousarray(x_real.T, dtype=bf)
    xi = np.ascontiguousarray(x_imag.T, dtype=bf)

    xsum = np.ascontiguousarray(
        (x_real.astype(np.float32) + x_imag.astype(np.float32)).T, dtype=bf)

    def pack3d(a):      # [512, BC] -> [128, NB, ND*BN], contiguous per b
        return np.ascontiguousarray(
            a.reshape(ND, P, NB, BN).transpose(1, 2, 0, 3).reshape(
                P, NB, ND * BN))

    in_maps = []
    for c in range(NCORES):
        sl = slice(c * BC, (c + 1) * BC)
        in_maps.append({
            "xT_r": pack3d(xr[:, sl]),
            "xT_i": pack3d(xi[:, sl]),
            "xT_s": pack3d(xsum[:, sl]),
            "F_1": F1, "F_2": F2, "F_3": F3,
            "W_1": W1, "W_2": W2,
        })
    return in_maps


def postprocess(results):
    outs = []
    for c in range(NCORES):
        z = results[c]["out"]
        mag = np.sqrt(z[:C] ** 2 + z[C:2 * C] ** 2).T
        outs.append(mag)
    return np.ascontiguousarray(np.concatenate(outs, axis=0), dtype=np.float32)


def kernel(x_real, x_imag, w0_real, w0_imag, wlast_real, wlast_imag):
    x_real, x_imag, w0_real, w0_imag, wlast_real, wlast_imag = (
        np.asarray(arr) for arr in
        (x_real, x_imag, w0_real, w0_imag, wlast_real, wlast_imag))
    nc = build_nc()
    in_maps = make_in_maps(
        x_real, x_imag, w0_real, w0_imag, wlast_real, wlast_imag)
    # A stale/wedged NeuronCore (e.g. a previously killed process that died
    # mid-execute) fails with NRT_EXEC_UNIT_UNRECOVERABLE; reloading resets
    # it but may need a fresh backend and a moment. Retry a few times.
    import time
    last = None
    for attempt in range(4):
        try:
            res = run_bass_kernel_spmd(
                nc, in_maps, core_ids=list(range(NCORES)))
            return postprocess(res.results)
        except Exception as e:
            last = e
            time.sleep(2.0 + 2.0 * attempt)
            try:
                import jax
                import jax.extend.backend
                jax.clear_caches()
                jax.extend.backend.clear_backends()
            except Exception:
                pass
    raise last

